# revision 26
# baseline (speedup 1.0000x reference)
"""Trainium2 Bass kernel for nn_DecoderTrans (transformer decoder layer + vocab head).

Sharding: 8 cores = (batch b, half hf). Each core computes the full trunk for its
512 "own" tokens (queries, key slots [512,1024)) and K/V context for the whole
1024-token sequence of its batch element. Trunk activations are feature-major
(x^T: [D, tokens]) in bf16. Masking is done by zeroing V rows (prefix half /
padded keys) and multiplying exp'd scores by 0/1 causal-diagonal masks on the
vector engine; exps are bias-free and batched two key-chunks at a time so the
scalar engine runs a pure-Exp stream in attention phases (no activation-table
reloads). Softmax/layernorm reciprocals use the single-op DVE approximation
(reciprocal_approx_fast) instead of the 3.3us table reciprocal. K/Q (resp. cQ)
projections are interleaved between attention head pairs as PE filler while
exps stream. All weight/constant loads are single batched DMAs so the sync
queue clears in ~10 issues; the vocab projection streams Wout in bf16 (one DMA
per 2000-column group, prefetched via pool rotation) while logit tiles DMA out
on the activation-engine queue.
"""
import math
import sys

sys.path.insert(0, "/opt/trn_rl_repo")

import numpy as np

import concourse.bass as bass
import concourse.tile as tile
from concourse import bacc, mybir
from concourse.bass import ts
from concourse.masks import make_identity

P = 128
D = 512
DC = D // P          # 4 feature chunks
T = 1024             # full sequence (keys)
TOWN = 512           # own tokens per core (queries), slots [512, 1024)
H = 8
DKH = 64             # head dim
V = 32000
VCH = 500            # vocab columns per psum bank
VG = 4               # psum banks per vocab group -> 2000 cols/group
NVG = V // (VCH * VG)  # 16 groups
FFN = 2 * D
SQRT_D = math.sqrt(D)
PAD_ID = 0

F32 = mybir.dt.float32
F32R = mybir.dt.float32r
BF16 = mybir.dt.bfloat16
I32 = mybir.dt.int32
AF = mybir.ActivationFunctionType
OP = mybir.AluOpType

WVBUFS = 4           # Wout group prefetch depth (tiles of [P,DC,2000] bf16)

# bias-pack column offsets in bpk [P, 36]
BOFF = {"bq": 0, "bk": 4, "bo1": 8, "cbq": 12, "ebk": 16, "bo2": 20,
        "b2": 24, "b1": 28}


# --------------------------------------------------------------------------
# program builder
# --------------------------------------------------------------------------

def build_module(flags):
    nc = bacc.Bacc("TRN2", target_bir_lowering=False, debug=False)

    def din(name, shape, dt=F32):
        return nc.dram_tensor(name, shape, dt, kind="ExternalInput").ap()

    a = {}
    a["idx"] = din("idx", [T, 1], I32)
    a["emb"] = din("emb", [V, D])
    a["peT"] = din("peT", [P, DC, T], BF16)
    a["encT"] = din("encT", [P, DC, T], BF16)
    a["masks"] = din("masks", [P, 4, TOWN], BF16)   # 0/1 causal diag
    a["bpk"] = din("bpk", [P, 36])                  # bias cols + b1
    a["pad01"] = din("pad01", [P, 8])               # per key-tile keep mask
    a["gpk"] = din("gpk", [1, 3 * D], F32R)         # LN gain rows
    for nm in ("WqT", "WkT", "WvT", "Wo1T", "cWqT", "eWkT", "eWvT", "Wo2T"):
        a[nm] = din(nm, [P, DC, D], BF16)
    a["W1T"] = din("W1T", [P, DC, FFN], BF16)
    a["W2T"] = din("W2T", [P, FFN // P, D], BF16)
    a["WoutT"] = din("WoutT", [P, DC, V], BF16)
    if flags["bias_v"]:
        a["bv_r"] = din("bv_r", [1, D], BF16)
        a["ebv_r"] = din("ebv_r", [1, D], BF16)
    for i in (1, 2, 3):
        if flags[f"ln_bias{i}"]:
            a[f"lb{i}_c"] = din(f"lb{i}_c", [P, DC])
    out = nc.dram_tensor("out", [TOWN, V], BF16, kind="ExternalOutput").ap()
    a["out"] = out
    import os
    if os.environ.get("KDBG") == "1":
        a["_dbg"] = {
            nm: nc.dram_tensor(f"dbg_{nm}", [P, TOWN], F32,
                               kind="ExternalOutput").ap()
            for nm in ("x0p0", "x0o0", "kT0a", "kT0b", "v0", "v4", "qT0",
                       "mgT0", "x1T0", "ekT0a", "ev0", "cqT0", "mg2T0",
                       "x2T0", "hT0", "x3T0")}
    else:
        a["_dbg"] = None

    with tile.TileContext(nc) as tc, \
         nc.allow_low_precision(reason="bf16 trunk"):
        _emit(tc, a, flags)
    nc.compile()
    return nc


def _emit(tc, a, flags):
    nc = tc.nc

    with tc.tile_pool(name="const", bufs=1) as cp, \
         tc.tile_pool(name="wts", bufs=1) as wtp, \
         tc.tile_pool(name="trunk", bufs=1) as trunkp, \
         tc.tile_pool(name="wD", bufs=WVBUFS) as wpv:

        # ---- startup-critical DMAs first (sync queue order = emission) ----
        idx_sb = cp.tile([P, 8], I32, tag="idx")
        nc.sync.dma_start(idx_sb[:], a["idx"].rearrange("(c p) o -> p (c o)", p=P))

        # ---- small constants (scalar queue + compute engines) ----
        ident = cp.tile([P, P], F32, tag="ident")
        make_identity(nc, ident[:])
        zscr = cp.tile([P, TOWN], F32, tag="zscr")
        nc.vector.memset(zscr[:], 0.0)
        ones_col = cp.tile([P, 1], BF16, tag="ones_col")
        nc.vector.tensor_scalar(ones_col[:], zscr[:, 0:1], 1.0, None, op0=OP.add)
        ones_row_r = cp.tile([1, P], F32R, tag="ones_row_r")
        nc.vector.tensor_scalar(ones_row_r[:], zscr[0:1, 0:P], 1.0, None,
                                op0=OP.add)
        eps_c = cp.tile([1, 1], F32, tag="eps_c")
        nc.vector.memset(eps_c[:], 1e-5)
        bpk = cp.tile([P, 36], F32, tag="bpk")
        nc.sync.dma_start(bpk[:], a["bpk"][:, :])
        pad01 = cp.tile([P, 8], F32, tag="pad01")
        nc.sync.dma_start(pad01[:], a["pad01"][:, :])
        masks = cp.tile([P, 4, TOWN], BF16, tag="masks")
        gpk = cp.tile([1, 3 * D], F32R, tag="gpk")
        lb_c = {}
        for i in (1, 2, 3):
            if flags[f"ln_bias{i}"]:
                lb_c[i] = cp.tile([P, DC], F32, tag=f"lb{i}_c", name=f"lb{i}_c")
                nc.sync.dma_start(lb_c[i][:], a[f"lb{i}_c"][:, :])
            else:
                lb_c[i] = None
        if flags["bias_v"]:
            bv_r = cp.tile([1, D], BF16, tag="bv_r")
            nc.sync.dma_start(bv_r[:], a["bv_r"][:, :])
            ebv_r = cp.tile([1, D], BF16, tag="ebv_r")
            nc.sync.dma_start(ebv_r[:], a["ebv_r"][:, :])
        else:
            bv_r = ebv_r = None

        def dbg(nm, ap):
            if a["_dbg"] is not None:
                t = cp.tile([P, TOWN], F32, tag="dbg", bufs=2, name=f"dbg{nm}")
                nc.vector.tensor_copy(t[:], ap)
                nc.sync.dma_start(a["_dbg"][nm][:, :], t[:])

        def bcol(nm, m):
            return bpk[:, BOFF[nm] + m: BOFF[nm] + m + 1]

        def grow(i, c):
            return gpk[0:1, (i - 1) * D + c * P: (i - 1) * D + c * P + P]

        def pe_keepalive(n):
            """Zero-dependency tensor-engine busy-work (stationary loads of a
            resident const) to hold the PE p-state through short stalls."""
            for _ in range(n):
                nc.tensor.ldweights(weights=masks[:, 0, 0:64])

        # ---- long-lived trunk activations ----
        x1T = [trunkp.tile([P, TOWN], BF16, tag=f"x1T{c}", name=f"x1T{c}")
               for c in range(DC)]
        x2T = [trunkp.tile([P, TOWN], BF16, tag=f"x2T{c}", name=f"x2T{c}")
               for c in range(DC)]
        x3T = [trunkp.tile([P, TOWN], BF16, tag=f"x3T{c}", name=f"x3T{c}")
               for c in range(DC)]

        # ================= shared helpers =================

        def vcopy(dst, ps, bias_ap, func=AF.Identity):
            """psum -> sbuf copy with bias (+relu) on the vector engine."""
            if func is AF.Relu:
                nc.vector.tensor_scalar(dst, ps, bias_ap, 0.0,
                                        op0=OP.add, op1=OP.max)
            else:
                nc.vector.tensor_scalar(dst, ps, bias_ap, None, op0=OP.add)

        def proj_chunk(dsts, src_halves, w, bnm, m, func=AF.Identity,
                       pbufs=2):
            for th in range(len(src_halves)):
                ps = pp.tile([P, TOWN], F32, tag="proj", bufs=pbufs)
                for c in range(DC):
                    nc.tensor.matmul(
                        ps[:], lhsT=w[:, c, ts(m, P)],
                        rhs=src_halves[th][c][:, :],
                        start=(c == 0), stop=(c == DC - 1))
                vcopy(dsts[m][:, th * TOWN:(th + 1) * TOWN], ps[:],
                      bcol(bnm, m), func)

        def proj_fm(dsts, src_halves, w, bnm, func=AF.Identity):
            for m in range(len(dsts)):
                proj_chunk(dsts, src_halves, w, bnm, m, func)

        def vproj(vtiles, src_slice, w, bias_row, pad_col_of, tok_range,
                  pbufs=2):
            """Row-major V projection: vtiles[t] [P, H*65]; col h*65+64 holds the
            keep-mask (1 normally, 0 for masked-out keys) so AV row 64
            accumulates the softmax denominator over kept keys only."""
            for t in tok_range:
                ps = pp.tile([P, D], F32, tag="proj", bufs=pbufs, name="vps")
                for c in range(DC):
                    nc.tensor.matmul(ps[:], lhsT=src_slice(c, t), rhs=w[:, c, :],
                                     start=(c == 0),
                                     stop=(bias_row is None and c == DC - 1))
                if bias_row is not None:
                    nc.tensor.matmul(ps[:], lhsT=ones_row_r[:], rhs=bias_row[:],
                                     start=False, stop=True)
                vt = vtiles[t]
                v3 = vt[:].rearrange("p (h e) -> p h e", e=65)
                ps3 = ps[:].rearrange("p (h e) -> p h e", e=64)
                pc = pad_col_of(t)
                if pc is not None:
                    nc.vector.tensor_scalar(v3[:, :, 0:64], ps3, pc, None,
                                            op0=OP.mult)
                    nc.vector.tensor_scalar(
                        v3[:, :, 64:65],
                        zscr[:, 0:8].rearrange("p (h e) -> p h e", e=1),
                        pc, None, op0=OP.add)
                else:
                    nc.vector.tensor_copy(v3[:, :, 0:64], ps3)
                    nc.vector.tensor_scalar(
                        v3[:, :, 64:65],
                        zscr[:, 0:8].rearrange("p (h e) -> p h e", e=1),
                        1.0, None, op0=OP.add)

        def _av_pair(av, vtiles, h, pi, pt):
            for i in range(2):
                kc = 2 * pi + i
                nc.tensor.matmul(
                    av[:], lhsT=vtiles[kc][:, h * 65: h * 65 + 65],
                    rhs=pt[:, i, :], start=(kc == 0), stop=(kc == 7))

        def attention(kT, vtiles, qT, use_masks, mergedT, pools, fillers=()):
            """Per head: scores (pairs of key chunks) -> batched exp -> AV,
            with AV emission delayed one pair so the PE stays dense. `fillers`
            is a queue of thunks (one proj matmul group each) popped into the
            pipeline slots between score/AV pairs so the PE never idles while
            the scalar engine streams exps; keep-alive stationary loads hold
            the PE p-state when the queue runs dry."""
            sp, avp, rp, sbp = pools
            fq = list(fillers)

            def slot():
                if fq:
                    fq.pop(0)()
                else:
                    pe_keepalive(2)

            for h in range(H):
                hc, off = h // 2, (h % 2) * DKH
                av = avp.tile([DKH + 1, TOWN], F32, tag="av", bufs=2)
                pts = []
                for pi in range(4):
                    s = sp.tile([P, 2, TOWN], F32, tag="s", bufs=2)
                    for i in range(2):
                        kc = 2 * pi + i
                        nc.tensor.matmul(
                            s[:, i, :], lhsT=kT[hc][off:off + DKH, ts(kc, P)],
                            rhs=qT[hc][off:off + DKH, :], start=True, stop=True)
                    pt = sbp.tile([P, 2, TOWN], BF16, tag="pt", bufs=4)
                    nc.scalar.activation(pt[:], s[:], AF.Exp, scale=0.125)
                    if use_masks and pi >= 2:
                        ptm = sbp.tile([P, 2, TOWN], BF16, tag="ptm", bufs=2)
                        nc.vector.tensor_tensor(
                            ptm[:], pt[:],
                            masks[:, (pi - 2) * 2:(pi - 2) * 2 + 2, :],
                            op=OP.mult)
                        pt = ptm
                    pts.append(pt)
                    slot()
                    if pi >= 1:
                        _av_pair(av, vtiles, h, pi - 1, pts[pi - 1])
                        slot()
                _av_pair(av, vtiles, h, 3, pts[3])
                slot()
                srow = sbp.tile([1, TOWN], F32, tag="srow", bufs=2)
                nc.scalar.copy(srow[:], av[DKH: DKH + 1, :])
                rrow_f = sbp.tile([1, TOWN], F32, tag="rrow_f", bufs=2)
                nc.vector.reciprocal_approx_fast(rrow_f[:], srow[:])
                rrow = sbp.tile([1, TOWN], F32R, tag="rrow", bufs=2)
                nc.vector.tensor_copy(rrow[:], rrow_f[:])
                R = rp.tile([DKH, TOWN], F32, tag="R")
                nc.tensor.matmul(R[:], lhsT=ones_row_r[:, 0:DKH], rhs=rrow[:],
                                 start=True, stop=True)
                Rs = sbp.tile([DKH, TOWN], BF16, tag="Rs", bufs=2)
                nc.vector.tensor_copy(Rs[:], R[:])
                nc.vector.tensor_tensor(mergedT[hc][off:off + DKH, :],
                                        av[0:DKH, :], Rs[:], op=OP.mult)

        def ln_stat_alloc(statp):
            return (statp.tile([1, TOWN], F32, tag="ssum", name="ssum"),
                    statp.tile([1, TOWN], F32, tag="ssq", name="ssq"))

        def ln_stat_accum(stats, sbp, li, c):
            """Accumulate ssum/ssq for chunk c right after li is produced."""
            ssum, ssq = stats
            nc.tensor.matmul(ssum[:], lhsT=ones_col[:], rhs=li[:],
                             start=(c == 0), stop=(c == DC - 1))
            sq = sbp.tile([P, TOWN], BF16, tag="sq", bufs=2)
            nc.vector.tensor_tensor(sq[:], li[:], li[:], op=OP.mult)
            nc.tensor.matmul(ssq[:], lhsT=ones_col[:], rhs=sq[:],
                             start=(c == 0), stop=(c == DC - 1))

        def layernorm(srcs, i, dsts, pools, stats=None):
            """dsts[c] = g*(srcs-mu)/sigma + b, feature-major chunks.
            Scalar engine only runs the Sqrt; everything else is DVE/PE."""
            statp, bcp, sbp = pools
            if stats is None:
                stats = ln_stat_alloc(statp)
                for c in range(DC):
                    ln_stat_accum(stats, sbp, srcs[c], c)
            ssum, ssq = stats
            mu_r = sbp.tile([1, TOWN], F32R, tag="lnrow2", bufs=3, name="mu_r")
            nc.vector.tensor_scalar(mu_r[:], ssum[:], 1.0 / D, None, op0=OP.mult)
            musq = sbp.tile([1, TOWN], F32, tag="lnrow2", bufs=3, name="musq")
            nc.vector.tensor_tensor(musq[:], mu_r[:], mu_r[:], op=OP.mult)
            var = sbp.tile([1, TOWN], F32, tag="lnrow2", bufs=3, name="var")
            nc.vector.scalar_tensor_tensor(
                var[:], in0=ssq[:], scalar=1.0 / D, in1=musq[:],
                op0=OP.mult, op1=OP.subtract)
            std = sbp.tile([1, TOWN], F32, tag="lnrow2", bufs=3, name="std")
            nc.scalar.activation(std[:], var[:], AF.Sqrt, bias=eps_c[:],
                                 scale=1.0)
            istd_f = sbp.tile([1, TOWN], F32, tag="lnrow2", bufs=3, name="istd_f")
            nc.vector.reciprocal_approx_fast(istd_f[:], std[:])
            istd = sbp.tile([1, TOWN], F32R, tag="lnrow2", bufs=3, name="istd")
            nc.vector.tensor_copy(istd[:], istd_f[:])
            pe_keepalive(24)
            mu_b = bcp.tile([P, TOWN], F32, tag="mu_b", bufs=1)
            nc.tensor.matmul(mu_b[:], lhsT=ones_row_r[:], rhs=mu_r[:],
                             start=True, stop=True)
            for c in range(DC):
                G = bcp.tile([P, TOWN], F32, tag="G", bufs=2)
                nc.tensor.matmul(G[:], lhsT=grow(i, c), rhs=istd[:],
                                 start=True, stop=True)
                t1 = sbp.tile([P, TOWN], BF16, tag="lnt", bufs=2)
                nc.vector.tensor_tensor(t1[:], srcs[c][:], mu_b[:],
                                        op=OP.subtract)
                if lb_c[i] is None:
                    nc.vector.tensor_tensor(dsts[c][:], t1[:], G[:], op=OP.mult)
                else:
                    t2 = sbp.tile([P, TOWN], F32, tag="lnt2", bufs=2)
                    nc.vector.tensor_tensor(t2[:], t1[:], G[:], op=OP.mult)
                    nc.vector.tensor_scalar(dsts[c][:], t2[:],
                                            lb_c[i][:, c:c + 1], None,
                                            op0=OP.add)

        # ================= trunk =================
        with tc.tile_pool(name="blkB", bufs=1) as bB:
            ekT = [bB.tile([P, T], BF16, tag=f"ekT{c}", name=f"ekT{c}")
                   for c in range(DC)]
            evsb = [bB.tile([P, H * 65], BF16, tag=f"ev{t}", name=f"ev{t}")
                    for t in range(8)]
            cqT = [bB.tile([P, TOWN], BF16, tag=f"cqT{c}", name=f"cqT{c}")
                   for c in range(DC)]
            mergedT2 = [bB.tile([P, TOWN], BF16, tag=f"mg2T{c}", name=f"mg2T{c}")
                        for c in range(DC)]

            with tc.tile_pool(name="blkA", bufs=1) as bA:
                x0p = [bA.tile([P, TOWN], BF16, tag=f"x0p{c}", name=f"x0p{c}")
                       for c in range(DC)]
                x0o = [bA.tile([P, TOWN], BF16, tag=f"x0o{c}", name=f"x0o{c}")
                       for c in range(DC)]
                kT = [bA.tile([P, T], BF16, tag=f"kT{c}", name=f"kT{c}")
                      for c in range(DC)]
                vsb = [bA.tile([P, H * 65], BF16, tag=f"v{t}", name=f"v{t}")
                       for t in range(8)]
                qT = [bA.tile([P, TOWN], BF16, tag=f"qT{c}", name=f"qT{c}")
                      for c in range(DC)]
                mergedT = [bA.tile([P, TOWN], BF16, tag=f"mgT{c}", name=f"mgT{c}")
                           for c in range(DC)]

                def x0slice(c, t):
                    return (x0p[c][:, ts(t, P)] if t < 4
                            else x0o[c][:, ts(t - 4, P)])

                # --- eK/eV from host-transposed encT + embedding gather ---
                with tc.tile_pool(name="early", bufs=1) as ep:
                  encT = ep.tile([P, DC, T], BF16, tag="encT")
                  nc.sync.dma_start(encT[:], a["encT"][:, :, :])
                  peT_sb = ep.tile([P, DC, T], BF16, tag="peT")
                  nc.sync.dma_start(peT_sb[:], a["peT"][:, :, :])

                  def load_w(nm, nf):
                      w = wtp.tile([P, DC, nf], BF16, tag=nm, name=nm)
                      nc.sync.dma_start(w[:], a[nm][:, :, :])
                      return w
                  w_ek = load_w("eWkT", D)
                  w_ev = load_w("eWvT", D)
                  w_k = load_w("WkT", D)
                  w_v = load_w("WvT", D)
                  w_q = load_w("WqT", D)
                  w_o1 = load_w("Wo1T", D)
                  w_cq = load_w("cWqT", D)
                  w_o2 = load_w("Wo2T", D)
                  nc.sync.dma_start(masks[:], a["masks"][:, :, :])
                  nc.sync.dma_start(gpk[:], a["gpk"][:, :])

                  def encslice(c, t):
                      return encT[:, c, ts(t, P)]

                  # --- embedding gather (bf16 cast) + transpose + pe ---
                  with tc.tile_pool(name="gath", bufs=3) as gp, \
                       tc.tile_pool(name="psT", bufs=4, space="PSUM") as tpp:
                    for t in range(8):
                        xg = gp.tile([P, D], F32, tag="xg")
                        nc.gpsimd.indirect_dma_start(
                            out=xg[:], out_offset=None, in_=a["emb"][:, :],
                            in_offset=bass.IndirectOffsetOnAxis(
                                ap=idx_sb[:, t: t + 1], axis=0))
                        for c in range(DC):
                            tp = tpp.tile([P, P], F32, tag="tp")
                            nc.tensor.transpose(tp[:], xg[:, ts(c, P)],
                                                ident[:])
                            nc.vector.scalar_tensor_tensor(
                                x0slice(c, t), in0=tp[:], scalar=SQRT_D,
                                in1=peT_sb[:, c, ts(t, P)],
                                op0=OP.mult, op1=OP.add)

                  with tc.tile_pool(name="psE", bufs=2, space="PSUM") as pp:
                    for th in range(2):
                        for m in range(DC):
                            ps = pp.tile([P, TOWN], F32, tag="proj", bufs=2)
                            for c in range(DC):
                                nc.tensor.matmul(
                                    ps[:], lhsT=w_ek[:, c, ts(m, P)],
                                    rhs=encT[:, c, th * TOWN:(th + 1) * TOWN],
                                    start=(c == 0), stop=(c == DC - 1))
                            vcopy(ekT[m][:, th * TOWN:(th + 1) * TOWN], ps[:],
                                  bcol("ebk", m))
                    vproj(evsb, encslice, w_ev, ebv_r, (lambda t: None),
                          range(8))


                dbg("x0p0", x0p[0][:])
                dbg("x0o0", x0o[0][:])
                dbg("ekT0a", ekT[0][:, 0:TOWN])
                dbg("ev0", evsb[0][:, 0:TOWN])

                # --- V projection (own psum scope, deep bufs) ---
                with tc.tile_pool(name="psV", bufs=4, space="PSUM") as pp:
                    vproj(vsb, x0slice, w_v, bv_r,
                          (lambda t: pad01[:, t:t + 1]), range(8), pbufs=4)
                    proj_chunk(kT, [x0p, x0o], w_k, "bk", 0, pbufs=4)
                    proj_chunk(qT, [x0o], w_q, "bq", 0, pbufs=4)

                # --- self-attention with K/Q interleaved ---
                with tc.tile_pool(name="psSA", bufs=1, space="PSUM") as pp, \
                     tc.tile_pool(name="psAV", bufs=1, space="PSUM") as avp, \
                     tc.tile_pool(name="psR", bufs=1, space="PSUM") as rp, \
                     tc.tile_pool(name="sbA", bufs=3) as sbp:

                    def kq_thunk(m, th):
                        def f():
                            ps = pp.tile([P, TOWN], F32, tag="proj", bufs=1)
                            for c in range(DC):
                                nc.tensor.matmul(
                                    ps[:], lhsT=w_k[:, c, ts(m, P)],
                                    rhs=(x0p if th == 0 else x0o)[c][:, :],
                                    start=(c == 0), stop=(c == DC - 1))
                            vcopy(kT[m][:, th * TOWN:(th + 1) * TOWN], ps[:],
                                  bcol("bk", m))
                        return f

                    def q_thunk(m):
                        def f():
                            proj_chunk(qT, [x0o], w_q, "bq", m, pbufs=1)
                        return f

                    fillers = []
                    for m in range(1, DC):
                        fillers += [kq_thunk(m, 0), kq_thunk(m, 1), q_thunk(m)]

                    attention(kT, vsb, qT, True, mergedT, (pp, avp, rp, sbp),
                              fillers=fillers)

                dbg("kT0a", kT[0][:, 0:TOWN])
                dbg("kT0b", kT[0][:, TOWN:T])
                dbg("v0", vsb[0][:, 0:TOWN])
                dbg("v4", vsb[4][:, 0:TOWN])
                dbg("qT0", qT[0][:])
                dbg("mgT0", mergedT[0][:])

                # --- Wo1 + residual + LN1 -> x1T ---
                with tc.tile_pool(name="psA2", bufs=2, space="PSUM") as pp2, \
                     tc.tile_pool(name="psStat", bufs=1, space="PSUM") as statp, \
                     tc.tile_pool(name="psBC", bufs=1, space="PSUM") as bcp, \
                     tc.tile_pool(name="sbLN", bufs=3) as sbp:
                    ln_in = []
                    stats = ln_stat_alloc(statp)
                    for m in range(DC):
                        ps = pp2.tile([P, TOWN], F32, tag="proj")
                        for c in range(DC):
                            nc.tensor.matmul(ps[:], lhsT=w_o1[:, c, ts(m, P)],
                                             rhs=mergedT[c][:],
                                             start=(c == 0), stop=(c == DC - 1))
                        li = sbp.tile([P, TOWN], BF16, tag="li", bufs=4,
                                      name=f"li{m}")
                        nc.vector.scalar_tensor_tensor(
                            li[:], in0=ps[:], scalar=bcol("bo1", m),
                            in1=x0o[m][:], op0=OP.add, op1=OP.add)
                        ln_in.append(li)
                        ln_stat_accum(stats, sbp, li, m)
                    layernorm(ln_in, 1, x1T, (statp, bcp, sbp), stats=stats)

            # --- cross-attention with cQ interleaved ---
            with tc.tile_pool(name="psSA", bufs=1, space="PSUM") as pp, \
                 tc.tile_pool(name="psAV", bufs=1, space="PSUM") as avp, \
                 tc.tile_pool(name="psR", bufs=1, space="PSUM") as rp, \
                 tc.tile_pool(name="sbB", bufs=3) as sbp:

                proj_chunk(cqT, [x1T], w_cq, "cbq", 0, pbufs=1)

                def cq_thunk(m):
                    def f():
                        proj_chunk(cqT, [x1T], w_cq, "cbq", m, pbufs=1)
                    return f

                attention(ekT, evsb, cqT, False, mergedT2, (pp, avp, rp, sbp),
                          fillers=[cq_thunk(m) for m in range(1, DC)])

            dbg("x1T0", x1T[0][:])
            dbg("cqT0", cqT[0][:])
            dbg("mg2T0", mergedT2[0][:])

            with tc.tile_pool(name="psB2", bufs=2, space="PSUM") as pp2, \
                 tc.tile_pool(name="psStat", bufs=1, space="PSUM") as statp, \
                 tc.tile_pool(name="psBC", bufs=1, space="PSUM") as bcp, \
                 tc.tile_pool(name="sbLN", bufs=3) as sbp:
                ln_in = []
                stats = ln_stat_alloc(statp)
                for m in range(DC):
                    ps = pp2.tile([P, TOWN], F32, tag="proj")
                    for c in range(DC):
                        nc.tensor.matmul(ps[:], lhsT=w_o2[:, c, ts(m, P)],
                                         rhs=mergedT2[c][:],
                                         start=(c == 0), stop=(c == DC - 1))
                    li = sbp.tile([P, TOWN], BF16, tag=f"li{m}", bufs=1)
                    nc.vector.scalar_tensor_tensor(
                        li[:], in0=ps[:], scalar=bcol("bo2", m),
                        in1=x1T[m][:], op0=OP.add, op1=OP.add)
                    ln_in.append(li)
                    ln_stat_accum(stats, sbp, li, m)
                layernorm(ln_in, 2, x2T, (statp, bcp, sbp), stats=stats)

        # ================= FFN + LN3 =================
        with tc.tile_pool(name="hC", bufs=1) as hp, \
             tc.tile_pool(name="psC", bufs=3, space="PSUM") as pp, \
             tc.tile_pool(name="psStat", bufs=1, space="PSUM") as statp, \
             tc.tile_pool(name="psBC", bufs=1, space="PSUM") as bcp, \
             tc.tile_pool(name="sbC", bufs=3) as sbp:
            w_1 = hp.tile([P, DC, FFN], BF16, tag="w1")
            nc.sync.dma_start(w_1[:], a["W1T"][:, :, :])
            w_2 = hp.tile([P, FFN // P, D], BF16, tag="w2")
            nc.sync.dma_start(w_2[:], a["W2T"][:, :, :])
            hT = [hp.tile([P, TOWN], BF16, tag=f"hT{m}", name=f"hT{m}")
                  for m in range(FFN // P)]
            for m in range(FFN // P):
                ps = pp.tile([P, TOWN], F32, tag="proj", bufs=2)
                for c in range(DC):
                    nc.tensor.matmul(ps[:], lhsT=w_1[:, c, ts(m, P)],
                                     rhs=x2T[c][:],
                                     start=(c == 0), stop=(c == DC - 1))
                vcopy(hT[m][:], ps[:], bcol("b1", m), AF.Relu)
            dbg("hT0", hT[0][:])
            ln_in = []
            stats = ln_stat_alloc(statp)
            for m in range(DC):
                ps = pp.tile([P, TOWN], F32, tag="proj", bufs=2)
                for c in range(FFN // P):
                    nc.tensor.matmul(ps[:], lhsT=w_2[:, c, ts(m, P)],
                                     rhs=hT[c][:],
                                     start=(c == 0), stop=(c == FFN // P - 1))
                li = sbp.tile([P, TOWN], BF16, tag=f"li{m}", bufs=1)
                nc.vector.scalar_tensor_tensor(
                    li[:], in0=ps[:], scalar=bcol("b2", m), in1=x2T[m][:],
                    op0=OP.add, op1=OP.add)
                ln_in.append(li)
                ln_stat_accum(stats, sbp, li, m)
            layernorm(ln_in, 3, x3T, (statp, bcp, sbp), stats=stats)

        if a["_dbg"] is not None:
            dbg("x2T0", x2T[0][:])
            dbg("x3T0", x3T[0][:])

        # ================= vocab projection (bf16) =================
        with tc.tile_pool(name="stD", bufs=4) as stp, \
             tc.tile_pool(name="psD", bufs=2, space="PSUM") as pp:
            for vg in range(NVG):
                cols = ts(vg, VG * VCH)
                w = wpv.tile([P, DC, VG * VCH], BF16, tag="wo")
                nc.sync.dma_start(w[:], a["WoutT"][:, :, cols])
                for t in range(TOWN // P):
                    ps = pp.tile([P, VG, 512], F32, tag="vps")
                    for j in range(VG):
                        for c in range(DC):
                            nc.tensor.matmul(
                                ps[:, j, 0:VCH], lhsT=x3T[c][:, ts(t, P)],
                                rhs=w[:, c, ts(j, VCH)],
                                start=(c == 0), stop=(c == DC - 1))
                    stage = stp.tile([P, VG, VCH], BF16, tag="stage")
                    if t == 3:
                        nc.scalar.copy(stage[:], ps[:, :, 0:VCH])
                    else:
                        nc.vector.tensor_copy(stage[:], ps[:, :, 0:VCH])
                    nc.scalar.dma_start(a["out"][ts(t, P), cols], stage[:])


# --------------------------------------------------------------------------
# host-side input preparation
# --------------------------------------------------------------------------

def _pos_encoding_np(t, d):
    pos = np.arange(t, dtype=np.float32)[:, None]
    freqs = 1.0 / (10000.0 ** (np.arange(0, d, 2, dtype=np.float32) / d))
    pe = np.zeros((t, d), np.float32)
    pe[:, 0::2] = np.sin(pos * freqs)
    pe[:, 1::2] = np.cos(pos * freqs)
    return pe


def _col_pack(b):
    """[n] -> [P, n//P] with element (p, c) = b[c*P + p]."""
    b = np.asarray(b, np.float32)
    return np.ascontiguousarray(b.reshape(-1, P).T)


def _wpack(m, bf):
    """[N_out, D_in] -> [P, DCin, N_out]: (p, c, n) = m[n, c*P + p]."""
    mT = np.asarray(m, np.float32).T            # [D_in, N_out]
    nin = mT.shape[0]
    return np.ascontiguousarray(
        mT.reshape(nin // P, P, -1).transpose(1, 0, 2)).astype(bf)


def make_flags(inputs):
    gi = lambda n: np.asarray(inputs[n])
    return {
        "bias_v": bool(np.any(gi("bv")) or np.any(gi("ebv"))),
        "ln_bias1": bool(np.any(gi("be1"))),
        "ln_bias2": bool(np.any(gi("be3"))),
        "ln_bias3": bool(np.any(gi("be2"))),
    }


def prep_in_maps(inputs, flags):
    import ml_dtypes
    BF = ml_dtypes.bfloat16
    gi = lambda n: np.asarray(inputs[n])
    tokens = gi("tokens").astype(np.int32)                      # [4, 1024]
    enc_all = gi("enc_embeddings").astype(np.float32)           # [4, 1024, 512]
    emb = np.ascontiguousarray(gi("emb").astype(np.float32))

    shared = {"emb": emb}
    for nm in ("Wq", "Wk", "Wv", "Wo1", "cWq", "eWk", "eWv", "Wo2", "W1", "W2",
               "Wout"):
        shared[nm + "T"] = _wpack(gi(nm), BF)

    bpk = np.zeros((P, 36), np.float32)
    for nm in ("bq", "bk", "bo1", "cbq", "ebk", "bo2", "b2", "b1"):
        cp = _col_pack(gi(nm))
        bpk[:, BOFF[nm]:BOFF[nm] + cp.shape[1]] = cp
    shared["bpk"] = bpk

    gpkv = np.concatenate([gi(g).astype(np.float32).reshape(D)
                           for g in ("g1", "g3", "g2")]).reshape(1, 3 * D)
    shared["gpk"] = np.ascontiguousarray(gpkv)
    for i, b in ((1, "be1"), (2, "be3"), (3, "be2")):
        if flags[f"ln_bias{i}"]:
            shared[f"lb{i}_c"] = _col_pack(gi(b).astype(np.float32))
    if flags["bias_v"]:
        shared["bv_r"] = gi("bv").astype(np.float32).reshape(1, D).astype(BF)
        shared["ebv_r"] = gi("ebv").astype(np.float32).reshape(1, D).astype(BF)

    # causal diagonal 0/1 keep-masks for key chunks 4..7 (slot space)
    kk = np.arange(P)[:, None]
    qq = np.arange(TOWN)[None, :]
    masks = np.zeros((P, 4, TOWN), np.float32)
    for j in range(4):
        masks[:, j, :] = ((j * P + kk) <= qq).astype(np.float32)
    shared["masks"] = masks.astype(BF)

    pe = _pos_encoding_np(T, D)

    in_maps = []
    for core in range(8):
        b, hf = core // 2, core % 2
        own = tokens[b, hf * 512:(hf + 1) * 512]
        idx_full = np.concatenate([tokens[b, :512], own])        # [1024]
        pe_slots = np.concatenate([pe[:512], pe[hf * 512:(hf + 1) * 512]],
                                  axis=0)
        peT = np.ascontiguousarray(
            pe_slots.T.reshape(DC, P, T).transpose(1, 0, 2))
        encTc = np.ascontiguousarray(
            enc_all[b].T.reshape(DC, P, T).transpose(1, 0, 2)).astype(BF)
        # per key-tile keep mask: 0 for blocked prefix (hf=0) and padded keys
        keep = (idx_full != PAD_ID).astype(np.float32)
        if hf == 0:
            keep[:512] = 0.0
        m = dict(shared)
        m["idx"] = np.ascontiguousarray(idx_full.reshape(T, 1))
        m["peT"] = peT.astype(BF)
        m["encT"] = encTc
        m["pad01"] = np.ascontiguousarray(keep.reshape(8, P).T)
        in_maps.append(m)
    return in_maps


def assemble(results, inputs):
    full = np.empty((4, 1024, V), np.float32)
    for core in range(8):
        b, hf = core // 2, core % 2
        full[b, hf * 512:(hf + 1) * 512] = np.asarray(
            results[core]["out"]).astype(np.float32)
    bout = np.asarray(inputs["bout"], np.float32)
    if np.any(bout):
        full += bout[None, None, :]
    return full


# --------------------------------------------------------------------------
# public entry point
# --------------------------------------------------------------------------

def kernel(**inputs):
    from concourse.bass_utils import run_bass_kernel_spmd
    flags = make_flags(inputs)
    nc = build_module(flags)
    in_maps = prep_in_maps(inputs, flags)
    res = run_bass_kernel_spmd(nc, in_maps, core_ids=list(range(8)))
    return assemble(res.results, inputs)


if __name__ == "__main__":
    flags = {"bias_v": False, "ln_bias1": False, "ln_bias2": False,
             "ln_bias3": False}
    nc = build_module(flags)
    print("built ok")
